# revision 18
# baseline (speedup 1.0000x reference)
"""Mode-pooling kernel for Trainium2 (8 NeuronCores, SPMD).

Problem: x (64, 65536, 10) fp32 holding integer class labels 0..9.
Reference does a flat reinterpretation to (B=64, C=10, D=65536), takes
non-overlapping windows of K=8 along D, and emits the mode of each window
(ties -> smallest value).  In flat terms: the input is 41,943,040 fp32
values; every consecutive group of 8 produces one mode value; the flat
mode array (64, 10, 8192) is then emitted as transpose(1,2,0) reshaped to
(64, 8192, 10).

Device algorithm (per core, fully data-parallel over 1/8 of the flat
array): histogram over the 10 possible values.  For each value v:
  eq_v   = (x == v) + (15-v)/128        (fused tensor_scalar, fp32)
  key_v  = pool_avg(eq_v, window=8)     = (16*c_v + 15 - v) / 128
  M      = max(M, key_v)                (fused scalar_tensor_tensor)
Decode: v* = 15 - (int(128*M) & 15); mode = v*.  Exact in fp32.
"""

import sys

sys.path.insert(0, "/opt/trn_rl_repo")

import numpy as np

import concourse.bass as bass
import concourse.bacc as bacc
import concourse.mybir as mybir
from concourse.tile import TileContext
from concourse.bass_utils import run_bass_kernel_spmd

N_CORES = 8
TOTAL = 64 * 65536 * 10          # 41,943,040 flat elements
PER_CORE = TOTAL // N_CORES      # 5,242,880
P = 128                          # SBUF partitions
FD = PER_CORE // P               # 40,960 elements per partition
K = 8                            # window size
W = FD // K                      # 5,120 windows per partition
CHUNK = 4096                     # columns per processing chunk
WCHUNK = CHUNK // K              # 1,024 windows per chunk
NCHUNK = FD // CHUNK             # 5 chunks

AF = mybir.AluOpType
DT = mybir.dt


def _build():
    nc = bacc.Bacc()
    x = nc.dram_tensor("x", (P, FD), DT.float32, kind="ExternalInput")
    out = nc.dram_tensor("out", (P, W), DT.float32, kind="ExternalOutput")

    with TileContext(nc) as tc:
        with tc.tile_pool(name="xin", bufs=2) as xpool, \
             tc.tile_pool(name="work", bufs=2) as wpool, \
             tc.tile_pool(name="outp", bufs=2) as opool:
            for ci in range(NCHUNK):
                xt = xpool.tile([P, CHUNK], DT.float32, tag="x")
                nc.scalar.dma_start(out=xt[:], in_=x[:, ci * CHUNK:(ci + 1) * CHUNK])

                # single reader of the DMA tile: cast fp32 -> bf16 (DVE 2x)
                xb = wpool.tile([P, CHUNK], DT.bfloat16, tag="xb")
                nc.vector.tensor_copy(xb[:], xt[:])

                eq = wpool.tile([P, CHUNK], DT.bfloat16, tag="eq")
                m0 = wpool.tile([P, WCHUNK], DT.float32, tag="m0")
                m1 = wpool.tile([P, WCHUNK], DT.float32, tag="m1")
                cnt = wpool.tile([P, WCHUNK], DT.float32, tag="cnt")
                for v in range(10):
                    # eq = (x == v) in bf16 (4x mode)
                    nc.vector.tensor_scalar(
                        eq[:], xb[:], float(v), None, AF.is_equal,
                    )
                    # windowed sum -> c_v (fp32)
                    eq3 = eq[:].rearrange("p (w k) -> p w k", k=K)
                    if v == 0:
                        nc.vector.tensor_reduce(
                            m0[:], eq3, mybir.AxisListType.X, AF.add)
                        # fold in the tie-break bias for v=0
                        nc.vector.tensor_scalar(
                            m0[:], m0[:], 15.0 / 128.0, None, AF.add)
                    else:
                        nc.vector.tensor_reduce(
                            cnt[:], eq3, mybir.AxisListType.X, AF.add)
                        # running max of keys: max(c_v + (15-v)/128, prev)
                        src, dst = (m0, m1) if v % 2 == 1 else (m1, m0)
                        nc.vector.scalar_tensor_tensor(
                            dst[:], cnt[:], (15.0 - v) / 128.0, src[:],
                            AF.add, AF.max,
                        )
                mfin = m1  # v=9 is odd -> dst=m1
                mi = wpool.tile([P, WCHUNK], DT.int32, tag="mi")
                nc.vector.tensor_scalar(mi[:], mfin[:], 128.0, None, AF.mult)
                vneg = wpool.tile([P, WCHUNK], DT.int32, tag="vneg")
                nc.vector.tensor_scalar(vneg[:], mi[:], 15, None, AF.bitwise_and)
                of = opool.tile([P, WCHUNK], DT.float32, tag="of")
                nc.vector.tensor_scalar(of[:], vneg[:], -1, 15, AF.mult, AF.add)
                nc.scalar.dma_start(
                    out=out[:, ci * WCHUNK:(ci + 1) * WCHUNK], in_=of[:]
                )
    nc.finalize()
    return nc


_NC = None


def kernel(x: np.ndarray) -> np.ndarray:
    global _NC
    x = np.ascontiguousarray(x, dtype=np.float32)
    flat = x.reshape(-1)
    shards = flat.reshape(N_CORES, P, FD)
    if _NC is None:
        _NC = _build()
    in_maps = [{"x": shards[i]} for i in range(N_CORES)]
    res = run_bass_kernel_spmd(_NC, in_maps, core_ids=list(range(N_CORES)))
    m_flat = np.concatenate([r["out"].reshape(-1) for r in res.results])
    m = m_flat.reshape(64, 10, 8192)
    out = np.transpose(m, (1, 2, 0)).reshape(64, 8192, 10)
    return np.ascontiguousarray(out, dtype=np.float32)


if __name__ == "__main__":
    rng = np.random.default_rng(0)
    xt = rng.integers(0, 10, size=(64, 65536, 10)).astype(np.float32)
    out = kernel(xt)
    print(out.shape, out.dtype, out[0, :4, :4])


# revision 21
# speedup vs baseline: 4.3236x; 4.3236x over previous
"""Mode-pooling kernel for Trainium2 (8 NeuronCores, SPMD) — v2.

Problem: x (64, 65536, 10) fp32 holding integer class labels 0..9.  In flat
terms: 41,943,040 fp32 values; every consecutive group of 8 produces one
mode value (ties -> smallest); the flat mode array (64, 10, 8192) is
emitted as transpose(1,2,0) reshaped to (64, 8192, 10).

v2 algorithm (per core, 1/8 of the flat array = 655,360 windows):

Host lays out each core's shard so the 8 elements of a window sit in 8
consecutive SBUF partitions: partition p = 8*g + k holds element k of
window w = g*40960 + n (n = column).  The kernel then:

1. casts fp32 -> int16 (values 0..9) on ACT,
2. builds two "nibble pack" features with fused int16 tensor_scalars (4x):
     p1 = bf16-bits of 2^(4*min(x,4))   (counts of 0..3 + #{x>=4} packed)
     p2 = bf16-bits of 2^(4*(max(x,4)-4)) (counts of 5..9 + #{x<=4} packed)
   (an int16 value (e+127)<<7 *is* the bf16 bit pattern of 2^e),
3. PE matmuls with a block-diagonal ones matrix sum each feature over the
   8 partitions of every window: PSUM S1/S2 hold, per window, all ten
   counts packed 4 bits per value — exactly (sums <= 2^23, fp32-exact),
4. ACT drains PSUM to SBUF as int32,
5. DVE extracts the ten 4-bit counts and runs a fused max chain over keys
   K_v = 16*c_v + (15-v); mode = 15 - (K_max & 15).  c4 is recovered as
   #{x<=4} + #{x>=4} - 8.
"""

import sys

sys.path.insert(0, "/opt/trn_rl_repo")

import ml_dtypes
import numpy as np

import concourse.bass as bass
import concourse.bacc as bacc
import concourse.mybir as mybir
from concourse.tile import TileContext
from concourse.bass_utils import run_bass_kernel_spmd

N_CORES = 8
TOTAL = 64 * 65536 * 10          # 41,943,040 flat elements
PER_CORE = TOTAL // N_CORES      # 5,242,880
P = 128                          # SBUF partitions
K = 8                            # window size
NW = PER_CORE // K               # 655,360 windows per core
G = P // K                       # 16 window-groups per column
COLS = NW // G                   # 40,960 columns
CHUNK = 4096                     # columns per chunk
NCHUNK = COLS // CHUNK           # 10 chunks
NT = CHUNK // P                  # 32 col-tiles (128 wide) per chunk
WCHUNK = NT * G                  # 512 window-results per partition per chunk
W = NCHUNK * WCHUNK              # 5,120 outputs per partition

AF = mybir.AluOpType
DT = mybir.dt


def _build():
    nc = bacc.Bacc()
    x = nc.dram_tensor("x", (P, COLS), DT.float32, kind="ExternalInput")
    out = nc.dram_tensor("out", (P, W), DT.float32, kind="ExternalOutput")

    with TileContext(nc) as tc:
        with tc.tile_pool(name="const", bufs=1) as cpool, \
             tc.tile_pool(name="xin", bufs=2) as xpool, \
             tc.tile_pool(name="work", bufs=2) as wpool, \
             tc.tile_pool(name="psum", bufs=2, space="PSUM") as ppool, \
             tc.tile_pool(name="outp", bufs=2) as opool:

            # block-diagonal ones (128, 16): ones[p, r] = (p // 8 == r)
            ones_np = np.zeros((P, G), dtype=np.float32)
            for r in range(G):
                ones_np[K * r:K * (r + 1), r] = 1.0
            ones_dram = nc.inline_tensor(
                ones_np.astype(ml_dtypes.bfloat16), name="ones_bd")
            ones = cpool.tile([P, G], DT.bfloat16, tag="ones")
            nc.scalar.dma_start(out=ones[:], in_=ones_dram[:])

            for ci in range(NCHUNK):
                xt = xpool.tile([P, CHUNK], DT.float32, tag="x")
                nc.scalar.dma_start(
                    out=xt[:], in_=x[:, ci * CHUNK:(ci + 1) * CHUNK])

                # fp32 -> int16 on ACT (single reader of the DMA tile)
                xi = wpool.tile([P, CHUNK], DT.int16, tag="xi")
                nc.scalar.copy(xi[:], xt[:])

                # pack features (all int16, DVE 4x)
                a = wpool.tile([P, CHUNK], DT.int16, tag="a")
                nc.vector.tensor_scalar(a[:], xi[:], 4, 4, AF.min, AF.mult)
                p1 = wpool.tile([P, CHUNK], DT.int16, tag="p1")
                nc.vector.tensor_scalar(p1[:], a[:], 127, 128, AF.add, AF.mult)
                b = wpool.tile([P, CHUNK], DT.int16, tag="b")
                nc.vector.tensor_scalar(b[:], xi[:], 4, 4, AF.max, AF.mult)
                p2 = wpool.tile([P, CHUNK], DT.int16, tag="p2")
                nc.vector.tensor_scalar(p2[:], b[:], 111, 128, AF.add, AF.mult)

                # windowed sums on PE: S[c, 16*t + r] = sum of group r, col c
                ps1 = ppool.tile([P, WCHUNK], DT.float32, tag="ps1")
                ps2 = ppool.tile([P, WCHUNK], DT.float32, tag="ps2")
                p1b = p1[:].bitcast(DT.bfloat16)
                p2b = p2[:].bitcast(DT.bfloat16)
                for t in range(NT):
                    nc.tensor.matmul(
                        ps1[:, G * t:G * (t + 1)],
                        p1b[:, P * t:P * (t + 1)], ones[:],
                        start=True, stop=True)
                for t in range(NT):
                    nc.tensor.matmul(
                        ps2[:, G * t:G * (t + 1)],
                        p2b[:, P * t:P * (t + 1)], ones[:],
                        start=True, stop=True)

                # drain PSUM -> int32 SBUF on ACT (values are exact ints)
                s1 = wpool.tile([P, WCHUNK], DT.int32, tag="s1")
                s2 = wpool.tile([P, WCHUNK], DT.int32, tag="s2")
                nc.scalar.copy(s1[:], ps1[:])
                nc.scalar.copy(s2[:], ps2[:])

                # ---- decode: max over keys K_v = 16*c_v + (15-v) ----
                e = wpool.tile([P, WCHUNK], DT.int32, tag="e")
                m0 = wpool.tile([P, WCHUNK], DT.int32, tag="m0")
                m1 = wpool.tile([P, WCHUNK], DT.int32, tag="m1")
                # v=0: K_0 = ((S1 << 4) & 0xF0) + 15
                nc.vector.tensor_scalar(
                    e[:], s1[:], 4, 0xF0, AF.logical_shift_left, AF.bitwise_and)
                nc.vector.tensor_scalar(m0[:], e[:], 15, None, AF.add)
                state = [m0, m1]

                def chain_step(ev, bias):
                    cur, nxt = state
                    nc.vector.scalar_tensor_tensor(
                        nxt[:], ev[:], bias, cur[:], AF.add, AF.max)
                    state.reverse()

                # v=1..3 from S1 nibbles 1..3
                for v in range(1, 4):
                    nc.vector.tensor_scalar(
                        e[:], s1[:], 4 * (v - 1), 0xF0,
                        AF.logical_shift_right, AF.bitwise_and)
                    chain_step(e, 15 - v)
                # c4 = A + A' - 8:  16*A = (S2 << 4) & 0xF0,
                #                   16*A' = (S1 >> 12) & 0xF0
                u = wpool.tile([P, WCHUNK], DT.int32, tag="u")
                nc.vector.tensor_scalar(
                    u[:], s2[:], 4, 0xF0, AF.logical_shift_left, AF.bitwise_and)
                w4 = wpool.tile([P, WCHUNK], DT.int32, tag="w4")
                nc.vector.tensor_scalar(
                    w4[:], s1[:], 12, 0xF0,
                    AF.logical_shift_right, AF.bitwise_and)
                # K_4 = 16*c4 + 11 = u + w4 - 117
                t4 = wpool.tile([P, WCHUNK], DT.int32, tag="t4")
                nc.vector.scalar_tensor_tensor(
                    t4[:], u[:], -117, w4[:], AF.add, AF.add)
                chain_step(t4, 0)
                # v=5..9 from S2 nibbles 1..5
                for v in range(5, 10):
                    nc.vector.tensor_scalar(
                        e[:], s2[:], 4 * (v - 5), 0xF0,
                        AF.logical_shift_right, AF.bitwise_and)
                    chain_step(e, 15 - v)

                # mode = 15 - (K_max & 15)
                vneg = wpool.tile([P, WCHUNK], DT.int32, tag="vneg")
                nc.vector.tensor_scalar(
                    vneg[:], state[0][:], 15, None, AF.bitwise_and)
                of = opool.tile([P, WCHUNK], DT.float32, tag="of")
                nc.vector.tensor_scalar(of[:], vneg[:], -1, 15, AF.mult, AF.add)
                nc.scalar.dma_start(
                    out=out[:, ci * WCHUNK:(ci + 1) * WCHUNK], in_=of[:])
    nc.finalize()
    return nc


_NC = None


def _shard_core(flat_core: np.ndarray) -> np.ndarray:
    # flat[w*8 + k] -> B[8*g + k, n] with w = g*COLS + n
    return np.ascontiguousarray(
        flat_core.reshape(G, COLS, K).transpose(0, 2, 1).reshape(P, COLS))


def kernel(x: np.ndarray) -> np.ndarray:
    global _NC
    x = np.ascontiguousarray(x, dtype=np.float32)
    flat = x.reshape(N_CORES, PER_CORE)
    if _NC is None:
        _NC = _build()
    in_maps = [{"x": _shard_core(flat[i])} for i in range(N_CORES)]
    res = run_bass_kernel_spmd(_NC, in_maps, core_ids=list(range(N_CORES)))
    # out[c, ci*512 + t*16 + r] = mode of window r*40960 + ci*4096 + t*128 + c
    parts = []
    for i in range(N_CORES):
        o = res.results[i]["out"].reshape(P, NCHUNK, NT, G)  # [c, ci, t, r]
        parts.append(o.transpose(3, 1, 2, 0).reshape(-1))    # w-major
    m_flat = np.concatenate(parts)
    m = m_flat.reshape(64, 10, 8192)
    outp = np.transpose(m, (1, 2, 0)).reshape(64, 8192, 10)
    return np.ascontiguousarray(outp, dtype=np.float32)


if __name__ == "__main__":
    rng = np.random.default_rng(0)
    xt = rng.integers(0, 10, size=(64, 65536, 10)).astype(np.float32)
    o = kernel(xt)
    print(o.shape, o.dtype)
